# revision 39
# baseline (speedup 1.0000x reference)
"""AttnBlock (B=2, C=512, H=W=64) on 8 TRN2 NeuronCores — fp8 DoubleRow.

Sharding: core c handles batch b=c//4 and query-quarter q=c%4 (1024 of 4096
query positions). Values are computed redundantly per core from the full
batch image. The key axis is host-permuted per core so the core's query
quarter occupies columns 0:1024 of its buffer — softmax/attention are
permutation-invariant over keys, so one SPMD program serves every core.

All heavy matmuls run in fp8 e4m3 with MatmulPerfMode.DoubleRow (256-deep
contraction, 0.5 cyc/row): x ships from the host in fp8, group-norm stats
run directly on the fp8 image, and the per-channel norm affine is folded
into the projection weights (w_eff = w * scale_c) plus a tiny on-device
bias matvec, so no normalize pass over x is needed.

The k projection is eliminated: S^T[j,i] = sum_c x[c,j] * qhat[c,i] with
qhat = scale_c * (Wk^T q) computed over the core's 1024 query columns
instead of 4096 key columns. The dropped terms (k bias, norm shift through
Wk) are per-query constants that cancel in softmax. exp outputs fp8 with a
-2 shift (also cancels) to stay inside e4m3 range. Z is reduced via tiny
bf16 matmuls (zsum stationary, ones moving) straight into per-partition
[q,1] layout. The residual path stays exact f32: the host bakes
x + bp + wp@bv into the residual tensor and the final projection applies
1/(16*32) to undo the fp8 scaling (weights x16, O x32).
"""

import numpy as np
import ml_dtypes

import concourse.bass as bass
import concourse.tile as tile
from concourse import bacc, mybir
from concourse.bass_utils import run_bass_kernel_spmd

F32 = mybir.dt.float32
F32R = mybir.dt.float32r
BF16 = mybir.dt.bfloat16
FP8 = mybir.dt.float8e4
DR = mybir.MatmulPerfMode.DoubleRow
NPFP8 = ml_dtypes.float8_e4m3

P = 128          # partitions
CT = 4           # channel tiles (C = 512 = 4*128)
C = 512
N = 4096         # H*W
NS = 8           # 512-wide column slices of N
NJT = 32         # 128-wide key tiles
NQ = 1024        # query columns per core
B = 2
HW = 64
NGROUPS = 32
GSIZE = C // NGROUPS  # 16 channels per group
EPS = 1e-5
SCL = float(C) ** -0.5
NCORES = 8
WS = 16.0        # weight scale (q, k, v, p weights shipped as 16*w)
OS = 32.0        # attention-output scale (osb = 32*O)
ESH = -2.0       # exp shift, cancels in softmax

_cached = {}


def _ct_layout(v):
    """[C] -> [P, CT] with channel c at [c % 128, c // 128]."""
    return np.ascontiguousarray(v.reshape(CT, P).T, dtype=np.float32)


def _cmaj(a2d, ncols):
    """[C, ncols] -> [P, CT, ncols]."""
    return np.ascontiguousarray(
        a2d.reshape(CT, P, ncols).transpose(1, 0, 2), dtype=np.float32
    )


def _build_program():
    nc = bacc.Bacc("TRN2", target_bir_lowering=False, debug=False)

    X_d = nc.declare_dram_parameter("xin", [P, NS, CT * 512], FP8, isOutput=False)
    XQ_d = nc.declare_dram_parameter("xqb", [P, CT, NQ], F32, isOutput=False)
    WQ_d = nc.declare_dram_parameter("wqt", [P, CT, C], FP8, isOutput=False)
    WK_d = nc.declare_dram_parameter("wk2", [P, CT, C], FP8, isOutput=False)
    WV_d = nc.declare_dram_parameter("wvt", [P, CT, C], FP8, isOutput=False)
    WP_d = nc.declare_dram_parameter("wpt", [P, CT, C], FP8, isOutput=False)
    CV_d = nc.declare_dram_parameter("cvec", [P, 3, CT], F32, isOutput=False)
    G_d = nc.declare_dram_parameter("gmat", [P, CT, NGROUPS], F32, isOutput=False)
    E_d = nc.declare_dram_parameter("emat", [NGROUPS, CT, P], F32, isOutput=False)
    ID_d = nc.declare_dram_parameter("idr", [P, P], F32, isOutput=False)
    OUT_d = nc.declare_dram_parameter("out", [P, CT, NQ], F32, isOutput=True)

    with tile.TileContext(nc) as tc:
        with (
            tc.tile_pool(name="big", bufs=1) as big,
            tc.tile_pool(name="consts", bufs=1) as consts,
            tc.tile_pool(name="stat", bufs=1) as stat,
        ):
            X = big.tile([P, NS, CT, 512], FP8)
            VT = big.tile([P, NJT, C], FP8)
            QO = big.tile([P, CT, NQ], FP8)
            QT = big.tile([P, CT, NQ], FP8)
            QP = big.tile([P, 2, 4, P], F32)
            XQB = big.tile([P, CT, NQ], F32)

            wq = consts.tile([P, CT, C], FP8)
            wk2 = consts.tile([P, CT, C], FP8)
            wv = consts.tile([P, CT, C], FP8)
            wppk = consts.tile([P, 4, C], FP8)
            wqs = consts.tile([P, CT, C], FP8)
            wvs = consts.tile([P, CT, C], FP8)
            cvec = consts.tile([P, 3, CT], F32)
            gmat = consts.tile([P, CT, NGROUPS], F32)
            emat = consts.tile([NGROUPS, CT, P], F32)
            idr = consts.tile([P, P], F32)
            onesf8 = consts.tile([P, 2, P], FP8)
            onef = consts.tile([1, 1], F32)
            neg2 = consts.tile([P, 1], F32)

            for s in range(NS):
                nc.sync.dma_start(out=X[:, s, :, :], in_=X_d[:, s, :])
            nc.sync.dma_start(out=wv, in_=WV_d[:])
            nc.sync.dma_start(out=wq, in_=WQ_d[:])
            nc.sync.dma_start(out=wk2, in_=WK_d[:])
            nc.sync.dma_start(out=wppk, in_=WP_d[:])
            nc.sync.dma_start(out=cvec, in_=CV_d[:])
            nc.sync.dma_start(out=gmat, in_=G_d[:])
            nc.sync.dma_start(out=emat, in_=E_d[:])
            nc.sync.dma_start(out=idr, in_=ID_d[:])
            for t in range(CT):
                nc.sync.dma_start(out=XQB[:, t, :], in_=XQ_d[:, t, :])
            nc.gpsimd.memset(onesf8, 1.0)
            nc.gpsimd.memset(onef, 1.0)
            nc.vector.memset(neg2, ESH)

            # ---------------- Phase 1: group-norm statistics ----------------
            # Split across DVE (bn_stats) and ACT (sum / sum-of-squares via
            # accum_out) so neither engine gates the startup alone.
            ACT_SLICES = (1, 3)
            DVE_SLICES = tuple(s for s in range(NS) if s not in ACT_SLICES)
            zero1 = consts.tile([P, 1], F32)
            nc.vector.memset(zero1, 0.0)
            bnst = stat.tile([P, CT, len(DVE_SLICES), 6], F32)
            sxg = stat.tile([P, CT, len(ACT_SLICES)], F32)
            sqa = stat.tile([P, CT, len(ACT_SLICES)], F32)
            scraps = [
                stat.tile([P, 512], F32, name=f"scrap{i}") for i in range(2)
            ]
            for s in range(NS):
                for t in range(CT):
                    if s in ACT_SLICES:
                        k = ACT_SLICES.index(s)
                        nc.scalar.activation(
                            out=scraps[0], in_=X[:, s, t, :],
                            func=mybir.ActivationFunctionType.Square,
                            bias=zero1, accum_out=sqa[:, t, k : k + 1],
                        )
                        nc.scalar.activation(
                            out=scraps[1], in_=X[:, s, t, :],
                            func=mybir.ActivationFunctionType.Copy,
                            bias=0.0, accum_out=sxg[:, t, k : k + 1],
                        )
                    else:
                        kd = DVE_SLICES.index(s)
                        nc.vector.bn_stats(
                            out=bnst[:, t, kd, :],
                            in_=X[:, s, t, :],
                        )
            mex = stat.tile([P, CT, 2], F32)
            for t in range(CT):
                nc.vector.bn_aggr(out=mex[:, t, :], in_=bnst[:, t, :, :])
            # combine: mexp0 = E[x], mexp1 = E[x^2] over all N columns
            FRD = 512.0 * len(DVE_SLICES) / N   # DVE sample fraction
            mexp = stat.tile([P, CT, 2], F32)
            sxt = stat.tile([P, CT, 2], F32)
            nc.vector.tensor_add(
                out=sxt[:, :, 0], in0=sxg[:, :, 0], in1=sxg[:, :, 1]
            )
            nc.vector.tensor_add(
                out=sxt[:, :, 1], in0=sqa[:, :, 0], in1=sqa[:, :, 1]
            )
            # E2_dve = var_dve + mean_dve^2
            nc.vector.tensor_tensor(
                out=mexp[:, :, 1], in0=mex[:, :, 0], in1=mex[:, :, 0],
                op=mybir.AluOpType.mult,
            )
            nc.vector.tensor_add(
                out=mexp[:, :, 1], in0=mexp[:, :, 1], in1=mex[:, :, 1]
            )
            # mexp0 = mean_dve*FRD + sx/N ; mexp1 = E2_dve*FRD + sxx/N
            nc.vector.tensor_scalar(
                out=mexp[:, :, 0], in0=mex[:, :, 0], scalar1=FRD,
                scalar2=None, op0=mybir.AluOpType.mult,
            )
            nc.vector.scalar_tensor_tensor(
                out=mexp[:, :, 0], in0=sxt[:, :, 0], scalar=1.0 / N,
                in1=mexp[:, :, 0],
                op0=mybir.AluOpType.mult, op1=mybir.AluOpType.add,
            )
            nc.vector.tensor_scalar(
                out=mexp[:, :, 1], in0=mexp[:, :, 1], scalar1=FRD,
                scalar2=None, op0=mybir.AluOpType.mult,
            )
            nc.vector.scalar_tensor_tensor(
                out=mexp[:, :, 1], in0=sxt[:, :, 1], scalar=1.0 / N,
                in1=mexp[:, :, 1],
                op0=mybir.AluOpType.mult, op1=mybir.AluOpType.add,
            )

            scale_c = stat.tile([P, CT], F32)
            scale16 = stat.tile([P, CT], F32)
            shift_c = stat.tile([P, CT], F32)
            shift256 = stat.tile([P, CT], FP8)
            with tc.tile_pool(name="psum_p1", bufs=1, space="PSUM") as p1:
                gs_ps = p1.tile([NGROUPS, 2], F32, tag="gs")
                for t in range(CT):
                    nc.tensor.matmul(
                        gs_ps, gmat[:, t, :], mexp[:, t, :],
                        start=(t == 0), stop=(t == CT - 1),
                    )
                gsb = stat.tile([NGROUPS, 2], F32)
                nc.vector.tensor_copy(out=gsb, in_=gs_ps)
                gmr = stat.tile([NGROUPS, 2], F32)
                gtmp = stat.tile([NGROUPS, 2], F32)
                nc.scalar.mul(out=gmr[:, 0:1], in_=gsb[:, 0:1], mul=1.0 / GSIZE)
                nc.scalar.mul(out=gtmp[:, 0:1], in_=gsb[:, 1:2], mul=1.0 / GSIZE)
                nc.vector.tensor_tensor(
                    out=gtmp[:, 1:2], in0=gmr[:, 0:1], in1=gmr[:, 0:1],
                    op=mybir.AluOpType.mult,
                )
                nc.vector.tensor_sub(
                    out=gtmp[:, 0:1], in0=gtmp[:, 0:1], in1=gtmp[:, 1:2]
                )
                eps_sb = stat.tile([NGROUPS, 1], F32)
                nc.vector.memset(eps_sb, EPS)
                nc.scalar.activation(
                    out=gtmp[:, 0:1], in_=gtmp[:, 0:1],
                    func=mybir.ActivationFunctionType.Sqrt, bias=eps_sb,
                )
                nc.vector.reciprocal(out=gmr[:, 1:2], in_=gtmp[:, 0:1])
                mc = stat.tile([P, CT, 2], F32)
                ms_ps = p1.tile([P, 8], F32, tag="ms")
                for t in range(CT):
                    nc.tensor.matmul(
                        ms_ps[:, 2 * t : 2 * t + 2], emat[:, t, :], gmr,
                        start=True, stop=True,
                    )
                nc.vector.tensor_copy(out=mc, in_=ms_ps)
                nc.vector.tensor_tensor(
                    out=scale_c, in0=mc[:, :, 1], in1=cvec[:, 1, :], op=mybir.AluOpType.mult
                )
                nc.vector.tensor_tensor(
                    out=shift_c, in0=mc[:, :, 0], in1=scale_c, op=mybir.AluOpType.mult
                )
                nc.vector.tensor_sub(out=shift_c, in0=cvec[:, 2, :], in1=shift_c)
                nc.gpsimd.tensor_scalar(
                    out=shift256, in0=shift_c, scalar1=256.0, scalar2=None,
                    op0=mybir.AluOpType.mult,
                )
                nc.gpsimd.tensor_scalar(
                    out=scale16, in0=scale_c, scalar1=1.0 / WS, scalar2=None,
                    op0=mybir.AluOpType.mult,
                )

            # fold norm scale into q/v weights: w_eff[c,o] = w16[c,o]*scale_c
            # (wvs first: the v projections are the first big PE block)
            for t in range(CT):
                nc.vector.tensor_scalar(
                    out=wvs[:, t, :], in0=wv[:, t, :],
                    scalar1=scale_c[:, t : t + 1], scalar2=None,
                    op0=mybir.AluOpType.mult,
                )
            for t in range(CT):
                nc.vector.tensor_scalar(
                    out=wqs[:, t, :], in0=wq[:, t, :],
                    scalar1=scale_c[:, t : t + 1], scalar2=None,
                    op0=mybir.AluOpType.mult,
                )

            # ---------------- Phase 2: q, qhat, v projections ---------------
            biasq = stat.tile([P, CT], F32)
            with (
                tc.tile_pool(name="psum2", bufs=1, space="PSUM") as psum2,
            ):
                # q bias matvec: biasq = (wq16^T @ shift256)/256 + 16*bq
                for ct in range(CT):
                    b_ps = psum2.tile([P, 1], F32, tag="bias", bufs=2)
                    for kt in range(0, CT, 2):
                        nc.tensor.matmul(
                            b_ps,
                            wq[:, kt : kt + 2, ct * P : (ct + 1) * P],
                            shift256[:, kt : kt + 2].unsqueeze(2),
                            start=(kt == 0), stop=(kt == 2),
                            perf_mode=DR,
                        )
                    nc.scalar.activation(
                        out=biasq[:, ct : ct + 1], in_=b_ps,
                        func=mybir.ActivationFunctionType.Identity,
                        scale=1.0 / 256.0, bias=cvec[:, 0, ct : ct + 1],
                    )

                def q_proj(s):
                    sl = slice(s * 512, (s + 1) * 512)
                    for ct in range(CT):
                        qp = psum2.tile([P, 512], F32, tag="acc", bufs=6)
                        for kt in range(0, CT, 2):
                            nc.tensor.matmul(
                                qp,
                                wqs[:, kt : kt + 2, ct * P : (ct + 1) * P],
                                X[:, s, kt : kt + 2, :],
                                start=(kt == 0), stop=(kt == 2),
                                perf_mode=DR,
                            )
                        nc.scalar.activation(
                            out=QO[:, ct, sl], in_=qp,
                            func=mybir.ActivationFunctionType.Identity,
                            bias=biasq[:, ct : ct + 1],
                        )

                def qhat_proj(isl):
                    """QT[c,i] = scale_c/16 * (wk16^T @ q16)[c,i] (fp8)."""
                    sl = slice(isl * 512, (isl + 1) * 512)
                    for ct in range(CT):
                        hp = psum2.tile([P, 512], F32, tag="acc", bufs=6)
                        for kt in range(0, CT, 2):
                            nc.tensor.matmul(
                                hp,
                                wk2[:, kt : kt + 2, ct * P : (ct + 1) * P],
                                QO[:, kt : kt + 2, sl],
                                start=(kt == 0), stop=(kt == 2),
                                perf_mode=DR,
                            )
                        nc.scalar.activation(
                            out=QT[:, ct, sl], in_=hp,
                            func=mybir.ActivationFunctionType.Copy,
                            scale=scale16[:, ct : ct + 1], bias=0.0,
                        )

                def v_slice(s):
                    # v^T: stationary = x block, moving = folded weights.
                    # Last slices' copies go to ACT (idle by then) so the
                    # DVE drain doesn't gate the phase-3 psum pool handoff.
                    for jt in range(CT):
                        vp = psum2.tile([P, 512], F32, tag="acc", bufs=6)
                        jcol = slice(jt * P, (jt + 1) * P)
                        for kt in range(0, CT, 2):
                            nc.tensor.matmul(
                                vp, X[:, s, kt : kt + 2, jcol], wvs[:, kt : kt + 2, :],
                                start=(kt == 0), stop=(kt == 2),
                                perf_mode=DR,
                            )
                        if s >= 6:
                            nc.scalar.activation(
                                out=VT[:, s * 4 + jt, :], in_=vp,
                                func=mybir.ActivationFunctionType.Copy,
                                bias=0.0,
                            )
                        else:
                            nc.vector.tensor_copy(
                                out=VT[:, s * 4 + jt, :], in_=vp
                            )

                # interleave: v-copies start draining on DVE while the PE
                # works through q/qhat, instead of queueing all 32 at the end
                v_slice(0)
                v_slice(1)
                q_proj(0)
                qhat_proj(0)
                v_slice(2)
                v_slice(3)
                q_proj(1)
                qhat_proj(1)
                for s in range(4, NS):
                    v_slice(s)

            # ---------------- Phase 3: attention (S^T route) -----------------
            with (
                tc.tile_pool(name="psum3", bufs=1, space="PSUM") as psum3,
                tc.tile_pool(name="pwork", bufs=1) as pwork,
            ):
                deferred = []

                def pop_deferred():
                    if deferred:
                        deferred.pop(0)()

                def st_group(isl, jt, pt2):
                    """S^T matmuls + exp for key tile jt into pt2[:, jt%2, :]."""
                    s_ps = psum3.tile([P, 512], F32, tag="s", bufs=2)
                    isl_sl = slice(isl * 512, (isl + 1) * 512)
                    js, jc = jt // 4, (jt % 4) * P
                    for kt in range(0, CT, 2):
                        nc.tensor.matmul(
                            s_ps,
                            X[:, js, kt : kt + 2, jc : jc + P],
                            QT[:, kt : kt + 2, isl_sl],
                            start=(kt == 0), stop=(kt == 2),
                            perf_mode=DR,
                        )
                    nc.scalar.activation(
                        out=pt2[:, jt % 2, :], in_=s_ps,
                        func=mybir.ActivationFunctionType.Exp,
                        scale=SCL / WS, bias=neg2,
                    )

                def emit_znorm(isl, z_ps, u_list):
                    """zinv per q-partition from the ones-matmul Z row.

                    Pipelined per ib block: reciprocal + normalize fire as
                    soon as that block's Z lands, so the first transpose can
                    start ~1us earlier in the tail.
                    """
                    zrow = pwork.tile([1, 512], F32, tag="zrow", bufs=2)
                    nc.vector.tensor_copy(out=zrow, in_=z_ps[0:1, :])
                    zvals = pwork.tile([P, 4], F32, tag="zv", bufs=2)
                    zinv = pwork.tile([P, 4], F32, tag="zi", bufs=2)
                    osbs = []
                    for ib in range(4):
                        zx = psum3.tile([P, 1], F32, tag="t", bufs=1)
                        nc.tensor.matmul(
                            zx, zrow[:, ib * P : (ib + 1) * P], onef,
                            start=True, stop=True,
                        )
                        nc.vector.tensor_copy(out=zvals[:, ib : ib + 1], in_=zx)
                        nc.vector.reciprocal(
                            out=zinv[:, ib : ib + 1], in_=zvals[:, ib : ib + 1]
                        )
                        osb = pwork.tile([P, C], FP8, tag="osb", bufs=4)
                        nc.vector.tensor_scalar(
                            out=osb, in0=u_list[ib],
                            scalar1=zinv[:, ib : ib + 1], scalar2=OS / WS,
                            op0=mybir.AluOpType.mult, op1=mybir.AluOpType.mult,
                        )
                        osbs.append(osb)
                    return osbs

                def otr_closures(isl, osbs):
                    """Deferred: packed-word transpose of O^T blocks into QP.

                    osb fp8 [q, 512c] viewed as f32r words [q, 128w] (word w
                    = channels 4w..4w+3); one transpose per ib block.
                    """
                    ops = []
                    for ib in range(4):
                        def otr(ib=ib):
                            t_ps = psum3.tile([P, P], F32, tag="t", bufs=1)
                            nc.tensor.transpose(
                                t_ps, osbs[ib].bitcast(F32), idr
                            )
                            nc.vector.tensor_copy(
                                out=QP[:, isl, ib, :], in_=t_ps
                            )

                        ops.append(otr)
                    return ops

                def proj_group(h, ob):
                    """Final projection for output block ob over i-slice h.

                    rhs: QP packed words re-read as fp8 [p, r, ib, q]; DR
                    pairs r in {0,1} and {2,3}; out cols ordered (ib, q) =
                    natural q. Then 1/512 unscale + residual + store.
                    """
                    sl = slice(h * 512, (h + 1) * 512)
                    pr = psum3.tile([P, 512], F32, tag="t", bufs=1)
                    rhs4 = QP[:, h, :, :].bitcast(FP8).rearrange(
                        "p ib (q r) -> p r ib q", r=4
                    )
                    for rp in range(2):
                        nc.tensor.matmul(
                            pr,
                            wppk[:, 2 * rp : 2 * rp + 2, ob * P : (ob + 1) * P],
                            rhs4[:, 2 * rp : 2 * rp + 2, :, :],
                            start=(rp == 0), stop=(rp == 1),
                            perf_mode=DR,
                        )
                    ost = pwork.tile([P, 512], F32, tag="ost", bufs=3)
                    for hh in range(2):
                        cs = slice(hh * 256, (hh + 1) * 256)
                        gs2 = slice(h * 512 + hh * 256, h * 512 + (hh + 1) * 256)
                        nc.vector.scalar_tensor_tensor(
                            out=ost[:, cs], in0=pr[:, cs], scalar=1.0 / (WS * OS),
                            in1=XQB[:, ob, gs2], op0=mybir.AluOpType.mult,
                            op1=mybir.AluOpType.add,
                        )
                        nc.sync.dma_start(out=OUT_d[:, ob, gs2], in_=ost[:, cs])

                for isl in range(2):
                    u_list = [
                        psum3.tile([P, C], F32, tag=f"u{ib}", bufs=1, name=f"u{ib}")
                        for ib in range(4)
                    ]
                    z_ps = psum3.tile([P, 512], F32, tag="z", bufs=1, name="z")

                    def pv_z(tpair, pair_pt2, u_list=u_list, z_ps=z_ps):
                        for ib in range(4):
                            nc.tensor.matmul(
                                u_list[ib],
                                pair_pt2[:, :, ib * P : (ib + 1) * P],
                                VT[:, 2 * tpair : 2 * tpair + 2, :],
                                start=(tpair == 0), stop=(tpair == 15),
                                perf_mode=DR,
                            )
                        nc.tensor.matmul(
                            z_ps, onesf8, pair_pt2,
                            start=(tpair == 0), stop=(tpair == 15),
                            perf_mode=DR,
                        )

                    pending = None  # deferred (tpair, pt2): PV trails the exp
                    pt2 = pwork.tile([P, 2, 512], FP8, tag="p", bufs=4)
                    st_group(isl, 0, pt2)
                    for jt in range(NJT):
                        cur_pt2 = pt2
                        if jt + 1 < NJT:
                            if jt % 2 == 1:
                                pt2 = pwork.tile([P, 2, 512], FP8, tag="p", bufs=4)
                            st_group(isl, jt + 1, pt2)
                        # PV + Z one pair behind, so exp(jt) hides under PE work
                        if jt % 2 == 1:
                            if pending is not None:
                                pv_z(*pending)
                            pending = (jt // 2, cur_pt2)
                        pop_deferred()
                        # i-slice 0's O is final once its 4 transposes popped
                        # (by jt=4 of isl 1) — run the h=0 projection here.
                        if isl == 1 and jt >= 5 and (jt - 5) % 8 == 0:
                            proj_group(0, (jt - 5) // 8)
                    pv_z(*pending)
                    osbs = emit_znorm(isl, z_ps, u_list)
                    deferred.extend(otr_closures(isl, osbs))

                # ---------------- Phase 4: remaining projection (h=1) --------
                while deferred:
                    pop_deferred()
                for ob in range(CT):
                    proj_group(1, ob)

    nc.compile()
    return nc


def _get_nc():
    if "nc" not in _cached:
        _cached["nc"] = _build_program()
    return _cached["nc"]


def _make_in_maps(x, norm_gamma, norm_beta, wq, bq, wk, bk, wv, bv, wp, bp):
    gm = np.zeros((P, CT, NGROUPS), np.float32)
    em = np.zeros((NGROUPS, CT, P), np.float32)
    for t in range(CT):
        for p in range(P):
            g = (t * P + p) // GSIZE
            gm[p, t, g] = 1.0
            em[g, t, p] = 1.0

    bpe = np.asarray(bp) + np.asarray(wp) @ np.asarray(bv)  # [C]
    # packed projection weights: wppk[cw, rr, o] = 16*wp[o, 4*cw+rr]
    wppk = np.ascontiguousarray(
        (np.asarray(wp).T * WS).reshape(P, 4, C), dtype=np.float32
    )
    common = {
        "wqt": _cmaj(np.asarray(wq).T * WS, C).astype(NPFP8),
        "wk2": _cmaj(np.asarray(wk) * WS, C).astype(NPFP8),
        "wvt": _cmaj(np.asarray(wv).T * WS, C).astype(NPFP8),
        "wpt": wppk.astype(NPFP8),
        "cvec": np.stack(
            [
                _ct_layout(np.asarray(bq) * WS),
                _ct_layout(np.asarray(norm_gamma)),
                _ct_layout(np.asarray(norm_beta)),
            ],
            axis=1,
        ),
        "gmat": gm,
        "emat": em,
        "idr": np.eye(P, dtype=np.float32),
    }

    in_maps = []
    for c in range(NCORES):
        b, qi = c // 4, c % 4
        xb = np.asarray(x[b], dtype=np.float32).reshape(C, N)
        xp = np.concatenate([xb[:, qi * NQ :], xb[:, : qi * NQ]], axis=1)
        m = dict(common)
        xcm = _cmaj(xp, N).reshape(P, CT, NS, 512)
        m["xin"] = np.ascontiguousarray(
            xcm.transpose(0, 2, 1, 3).reshape(P, NS, CT * 512)
        ).astype(NPFP8)
        m["xqb"] = _cmaj(
            xb[:, qi * NQ : (qi + 1) * NQ] + bpe[:, None], NQ
        )
        in_maps.append(m)
    return in_maps


def _assemble(results):
    out = np.empty((B, C, N), np.float32)
    for c in range(NCORES):
        b, qi = c // 4, c % 4
        r = results[c]["out"]  # [P, CT, NQ]
        out[b, :, qi * NQ : (qi + 1) * NQ] = (
            r.transpose(1, 0, 2).reshape(C, NQ)
        )
    return out.reshape(B, C, HW, HW)


def _run(inputs, trace=False, trace_kwargs=None):
    nc = _get_nc()
    in_maps = _make_in_maps(**inputs)
    res = run_bass_kernel_spmd(
        nc, in_maps, list(range(NCORES)), trace=trace,
        **(trace_kwargs or {}),
    )
    return res


def kernel(**inputs):
    res = _run(inputs)
    return _assemble(res.results)


# revision 41
# speedup vs baseline: 1.1780x; 1.1780x over previous
"""AttnBlock (B=2, C=512, H=W=64) on 8 TRN2 NeuronCores — fp8 DoubleRow.

Sharding: core c handles batch b=c//4 and query-quarter q=c%4 (1024 of 4096
query positions). Values are computed redundantly per core from the full
batch image. The key axis is host-permuted per core so the core's query
quarter occupies columns 0:1024 of its buffer — softmax/attention are
permutation-invariant over keys, so one SPMD program serves every core.

All heavy matmuls run in fp8 e4m3 with MatmulPerfMode.DoubleRow (256-deep
contraction, 0.5 cyc/row): x ships from the host in fp8, group-norm stats
run directly on the fp8 image, and the per-channel norm affine is folded
into the projection weights (w_eff = w * scale_c) plus a tiny on-device
bias matvec, so no normalize pass over x is needed.

The k projection is eliminated: S^T[j,i] = sum_c x[c,j] * qhat[c,i] with
qhat = scale_c * (Wk^T q) computed over the core's 1024 query columns
instead of 4096 key columns. The dropped terms (k bias, norm shift through
Wk) are per-query constants that cancel in softmax. exp outputs fp8 with a
-2 shift (also cancels) to stay inside e4m3 range. Z is reduced via tiny
bf16 matmuls (zsum stationary, ones moving) straight into per-partition
[q,1] layout. The residual path stays exact f32: the host bakes
x + bp + wp@bv into the residual tensor and the final projection applies
1/(16*32) to undo the fp8 scaling (weights x16, O x32).
"""

import numpy as np
import ml_dtypes

import concourse.bass as bass
import concourse.tile as tile
from concourse import bacc, mybir
from concourse.bass_utils import run_bass_kernel_spmd

F32 = mybir.dt.float32
F32R = mybir.dt.float32r
BF16 = mybir.dt.bfloat16
FP8 = mybir.dt.float8e4
DR = mybir.MatmulPerfMode.DoubleRow
NPFP8 = ml_dtypes.float8_e4m3

P = 128          # partitions
CT = 4           # channel tiles (C = 512 = 4*128)
C = 512
N = 4096         # H*W
NS = 8           # 512-wide column slices of N
NJT = 32         # 128-wide key tiles
NQ = 1024        # query columns per core
B = 2
HW = 64
NGROUPS = 32
GSIZE = C // NGROUPS  # 16 channels per group
EPS = 1e-5
SCL = float(C) ** -0.5
NCORES = 8
WS = 16.0        # weight scale (q, k, v, p weights shipped as 16*w)
OS = 32.0        # attention-output scale (osb = 32*O)
ESH = -2.0       # exp shift, cancels in softmax

_cached = {}


def _ct_layout(v):
    """[C] -> [P, CT] with channel c at [c % 128, c // 128]."""
    return np.ascontiguousarray(v.reshape(CT, P).T, dtype=np.float32)


def _cmaj(a2d, ncols):
    """[C, ncols] -> [P, CT, ncols]."""
    return np.ascontiguousarray(
        a2d.reshape(CT, P, ncols).transpose(1, 0, 2), dtype=np.float32
    )


def _build_program():
    nc = bacc.Bacc("TRN2", target_bir_lowering=False, debug=False)

    X_d = nc.declare_dram_parameter("xin", [P, NS, CT * 512], FP8, isOutput=False)
    XQ_d = nc.declare_dram_parameter("xqb", [P, CT, NQ], F32, isOutput=False)
    WQ_d = nc.declare_dram_parameter("wqt", [P, CT, C], FP8, isOutput=False)
    WK_d = nc.declare_dram_parameter("wk2", [P, CT, C], FP8, isOutput=False)
    WV_d = nc.declare_dram_parameter("wvt", [P, CT, C], FP8, isOutput=False)
    WP_d = nc.declare_dram_parameter("wpt", [P, CT, C], FP8, isOutput=False)
    CV_d = nc.declare_dram_parameter("cvec", [P, 3, CT], F32, isOutput=False)
    G_d = nc.declare_dram_parameter("gmat", [P, CT, NGROUPS], F32, isOutput=False)
    E_d = nc.declare_dram_parameter("emat", [NGROUPS, CT, P], F32, isOutput=False)
    ID_d = nc.declare_dram_parameter("idr", [P, P], F32, isOutput=False)
    OUT_d = nc.declare_dram_parameter("out", [P, CT, NQ], F32, isOutput=True)

    with tile.TileContext(nc) as tc:
        with (
            tc.tile_pool(name="big", bufs=1) as big,
            tc.tile_pool(name="consts", bufs=1) as consts,
            tc.tile_pool(name="stat", bufs=1) as stat,
        ):
            X = big.tile([P, NS, CT, 512], FP8)
            VT = big.tile([P, NJT, C], FP8)
            QO = big.tile([P, CT, NQ], FP8)
            QT = big.tile([P, CT, NQ], FP8)
            QP = big.tile([P, 2, 4, P], F32)
            XQB = big.tile([P, CT, NQ], F32)

            wq = consts.tile([P, CT, C], FP8)
            wk2 = consts.tile([P, CT, C], FP8)
            wv = consts.tile([P, CT, C], FP8)
            wppk = consts.tile([P, 4, C], FP8)
            wqs = consts.tile([P, CT, C], FP8)
            wvs = consts.tile([P, CT, C], FP8)
            cvec = consts.tile([P, 3, CT], F32)
            gmat = consts.tile([P, CT, NGROUPS], F32)
            emat = consts.tile([NGROUPS, CT, P], F32)
            idr = consts.tile([P, P], F32)
            onesf8 = consts.tile([P, 2, P], FP8)
            onef = consts.tile([1, 1], F32)
            neg2 = consts.tile([P, 1], F32)

            for s in range(NS):
                nc.sync.dma_start(out=X[:, s, :, :], in_=X_d[:, s, :])
            nc.sync.dma_start(out=wv, in_=WV_d[:])
            nc.sync.dma_start(out=wq, in_=WQ_d[:])
            nc.sync.dma_start(out=wk2, in_=WK_d[:])
            nc.sync.dma_start(out=wppk, in_=WP_d[:])
            nc.sync.dma_start(out=cvec, in_=CV_d[:])
            nc.sync.dma_start(out=gmat, in_=G_d[:])
            nc.sync.dma_start(out=emat, in_=E_d[:])
            nc.sync.dma_start(out=idr, in_=ID_d[:])
            for t in range(CT):
                nc.sync.dma_start(out=XQB[:, t, :], in_=XQ_d[:, t, :])
            nc.gpsimd.memset(onesf8, 1.0)
            nc.gpsimd.memset(onef, 1.0)
            nc.vector.memset(neg2, ESH)

            # ---------------- Phase 1: group-norm statistics ----------------
            # Split across DVE (bn_stats) and ACT (sum / sum-of-squares via
            # accum_out) so neither engine gates the startup alone.
            ACT_SLICES = (1, 3)
            DVE_SLICES = tuple(s for s in range(NS) if s not in ACT_SLICES)
            zero1 = consts.tile([P, 1], F32)
            nc.vector.memset(zero1, 0.0)
            bnst = stat.tile([P, CT, len(DVE_SLICES), 6], F32)
            sxg = stat.tile([P, CT, len(ACT_SLICES)], F32)
            sqa = stat.tile([P, CT, len(ACT_SLICES)], F32)
            scraps = [
                stat.tile([P, 512], F32, name=f"scrap{i}") for i in range(2)
            ]
            for s in range(NS):
                for t in range(CT):
                    if s in ACT_SLICES:
                        k = ACT_SLICES.index(s)
                        nc.scalar.activation(
                            out=scraps[0], in_=X[:, s, t, :],
                            func=mybir.ActivationFunctionType.Square,
                            bias=zero1, accum_out=sqa[:, t, k : k + 1],
                        )
                        nc.scalar.activation(
                            out=scraps[1], in_=X[:, s, t, :],
                            func=mybir.ActivationFunctionType.Copy,
                            bias=0.0, accum_out=sxg[:, t, k : k + 1],
                        )
                    else:
                        kd = DVE_SLICES.index(s)
                        nc.vector.bn_stats(
                            out=bnst[:, t, kd, :],
                            in_=X[:, s, t, :],
                        )
            mex = stat.tile([P, CT, 2], F32)
            for t in range(CT):
                nc.vector.bn_aggr(out=mex[:, t, :], in_=bnst[:, t, :, :])
            # combine: mexp0 = E[x], mexp1 = E[x^2] over all N columns
            FRD = 512.0 * len(DVE_SLICES) / N   # DVE sample fraction
            mexp = stat.tile([P, CT, 2], F32)
            sxt = stat.tile([P, CT, 2], F32)
            nc.vector.tensor_add(
                out=sxt[:, :, 0], in0=sxg[:, :, 0], in1=sxg[:, :, 1]
            )
            nc.vector.tensor_add(
                out=sxt[:, :, 1], in0=sqa[:, :, 0], in1=sqa[:, :, 1]
            )
            # E2_dve = var_dve + mean_dve^2
            nc.vector.tensor_tensor(
                out=mexp[:, :, 1], in0=mex[:, :, 0], in1=mex[:, :, 0],
                op=mybir.AluOpType.mult,
            )
            nc.vector.tensor_add(
                out=mexp[:, :, 1], in0=mexp[:, :, 1], in1=mex[:, :, 1]
            )
            # mexp0 = mean_dve*FRD + sx/N ; mexp1 = E2_dve*FRD + sxx/N
            nc.vector.tensor_scalar(
                out=mexp[:, :, 0], in0=mex[:, :, 0], scalar1=FRD,
                scalar2=None, op0=mybir.AluOpType.mult,
            )
            nc.vector.scalar_tensor_tensor(
                out=mexp[:, :, 0], in0=sxt[:, :, 0], scalar=1.0 / N,
                in1=mexp[:, :, 0],
                op0=mybir.AluOpType.mult, op1=mybir.AluOpType.add,
            )
            nc.vector.tensor_scalar(
                out=mexp[:, :, 1], in0=mexp[:, :, 1], scalar1=FRD,
                scalar2=None, op0=mybir.AluOpType.mult,
            )
            nc.vector.scalar_tensor_tensor(
                out=mexp[:, :, 1], in0=sxt[:, :, 1], scalar=1.0 / N,
                in1=mexp[:, :, 1],
                op0=mybir.AluOpType.mult, op1=mybir.AluOpType.add,
            )

            scale_c = stat.tile([P, CT], F32)
            scale16 = stat.tile([P, CT], F32)
            shift_c = stat.tile([P, CT], F32)
            shift256 = stat.tile([P, CT], FP8)
            with tc.tile_pool(name="psum_p1", bufs=1, space="PSUM") as p1:
                gs_ps = p1.tile([NGROUPS, 2], F32, tag="gs")
                for t in range(CT):
                    nc.tensor.matmul(
                        gs_ps, gmat[:, t, :], mexp[:, t, :],
                        start=(t == 0), stop=(t == CT - 1),
                    )
                gsb = stat.tile([NGROUPS, 2], F32)
                nc.vector.tensor_copy(out=gsb, in_=gs_ps)
                gmr = stat.tile([NGROUPS, 2], F32)
                gtmp = stat.tile([NGROUPS, 2], F32)
                nc.scalar.mul(out=gmr[:, 0:1], in_=gsb[:, 0:1], mul=1.0 / GSIZE)
                nc.scalar.mul(out=gtmp[:, 0:1], in_=gsb[:, 1:2], mul=1.0 / GSIZE)
                nc.vector.tensor_tensor(
                    out=gtmp[:, 1:2], in0=gmr[:, 0:1], in1=gmr[:, 0:1],
                    op=mybir.AluOpType.mult,
                )
                nc.vector.tensor_sub(
                    out=gtmp[:, 0:1], in0=gtmp[:, 0:1], in1=gtmp[:, 1:2]
                )
                eps_sb = stat.tile([NGROUPS, 1], F32)
                nc.vector.memset(eps_sb, EPS)
                nc.scalar.activation(
                    out=gtmp[:, 0:1], in_=gtmp[:, 0:1],
                    func=mybir.ActivationFunctionType.Sqrt, bias=eps_sb,
                )
                nc.vector.reciprocal(out=gmr[:, 1:2], in_=gtmp[:, 0:1])
                mc = stat.tile([P, CT, 2], F32)
                ms_ps = p1.tile([P, 8], F32, tag="ms")
                for t in range(CT):
                    nc.tensor.matmul(
                        ms_ps[:, 2 * t : 2 * t + 2], emat[:, t, :], gmr,
                        start=True, stop=True,
                    )
                nc.vector.tensor_copy(out=mc, in_=ms_ps)
                nc.vector.tensor_tensor(
                    out=scale_c, in0=mc[:, :, 1], in1=cvec[:, 1, :], op=mybir.AluOpType.mult
                )
                nc.vector.tensor_tensor(
                    out=shift_c, in0=mc[:, :, 0], in1=scale_c, op=mybir.AluOpType.mult
                )
                nc.vector.tensor_sub(out=shift_c, in0=cvec[:, 2, :], in1=shift_c)
                nc.gpsimd.tensor_scalar(
                    out=shift256, in0=shift_c, scalar1=256.0, scalar2=None,
                    op0=mybir.AluOpType.mult,
                )
                nc.gpsimd.tensor_scalar(
                    out=scale16, in0=scale_c, scalar1=1.0 / WS, scalar2=None,
                    op0=mybir.AluOpType.mult,
                )

            # fold norm scale into q/v weights: w_eff[c,o] = w16[c,o]*scale_c
            # (wvs first: the v projections are the first big PE block)
            for t in range(CT):
                nc.vector.tensor_scalar(
                    out=wvs[:, t, :], in0=wv[:, t, :],
                    scalar1=scale_c[:, t : t + 1], scalar2=None,
                    op0=mybir.AluOpType.mult,
                )
            for t in range(CT):
                nc.vector.tensor_scalar(
                    out=wqs[:, t, :], in0=wq[:, t, :],
                    scalar1=scale_c[:, t : t + 1], scalar2=None,
                    op0=mybir.AluOpType.mult,
                )

            # ---------------- Phase 2: q, qhat, v projections ---------------
            biasq = stat.tile([P, CT], F32)
            with (
                tc.tile_pool(name="psum2", bufs=1, space="PSUM") as psum2,
            ):
                def bias_mv():
                    # biasq = (wq16^T @ shift256)/256 + 16*bq
                    for ct in range(CT):
                        b_ps = psum2.tile([P, 1], F32, tag="bias", bufs=2)
                        for kt in range(0, CT, 2):
                            nc.tensor.matmul(
                                b_ps,
                                wq[:, kt : kt + 2, ct * P : (ct + 1) * P],
                                shift256[:, kt : kt + 2].unsqueeze(2),
                                start=(kt == 0), stop=(kt == 2),
                                perf_mode=DR,
                            )
                        nc.scalar.activation(
                            out=biasq[:, ct : ct + 1], in_=b_ps,
                            func=mybir.ActivationFunctionType.Identity,
                            scale=1.0 / 256.0, bias=cvec[:, 0, ct : ct + 1],
                        )

                def q_proj(s):
                    sl = slice(s * 512, (s + 1) * 512)
                    for ct in range(CT):
                        qp = psum2.tile([P, 512], F32, tag="acc", bufs=6)
                        for kt in range(0, CT, 2):
                            nc.tensor.matmul(
                                qp,
                                wqs[:, kt : kt + 2, ct * P : (ct + 1) * P],
                                X[:, s, kt : kt + 2, :],
                                start=(kt == 0), stop=(kt == 2),
                                perf_mode=DR,
                            )
                        nc.scalar.activation(
                            out=QO[:, ct, sl], in_=qp,
                            func=mybir.ActivationFunctionType.Identity,
                            bias=biasq[:, ct : ct + 1],
                        )

                def qhat_proj(isl):
                    """QT[c,i] = scale_c/16 * (wk16^T @ q16)[c,i] (fp8)."""
                    sl = slice(isl * 512, (isl + 1) * 512)
                    for ct in range(CT):
                        hp = psum2.tile([P, 512], F32, tag="acc", bufs=6)
                        for kt in range(0, CT, 2):
                            nc.tensor.matmul(
                                hp,
                                wk2[:, kt : kt + 2, ct * P : (ct + 1) * P],
                                QO[:, kt : kt + 2, sl],
                                start=(kt == 0), stop=(kt == 2),
                                perf_mode=DR,
                            )
                        nc.scalar.activation(
                            out=QT[:, ct, sl], in_=hp,
                            func=mybir.ActivationFunctionType.Copy,
                            scale=scale16[:, ct : ct + 1], bias=0.0,
                        )

                def v_slice(s):
                    # v^T: stationary = x block, moving = folded weights
                    for jt in range(CT):
                        vp = psum2.tile([P, 512], F32, tag="acc", bufs=6)
                        jcol = slice(jt * P, (jt + 1) * P)
                        for kt in range(0, CT, 2):
                            nc.tensor.matmul(
                                vp, X[:, s, kt : kt + 2, jcol], wvs[:, kt : kt + 2, :],
                                start=(kt == 0), stop=(kt == 2),
                                perf_mode=DR,
                            )
                        nc.vector.tensor_copy(
                            out=VT[:, s * 4 + jt, :], in_=vp
                        )

                # interleave: v-copies start draining on DVE while the PE
                # works through q/qhat; PE fill between every ACT round trip
                v_slice(0)
                bias_mv()
                v_slice(1)
                q_proj(0)
                q_proj(1)
                v_slice(2)
                v_slice(3)
                qhat_proj(0)
                qhat_proj(1)
                for s in range(4, NS):
                    v_slice(s)

            # ---------------- Phase 3: attention (S^T route) -----------------
            with (
                tc.tile_pool(name="psum3", bufs=1, space="PSUM") as psum3,
                tc.tile_pool(name="pwork", bufs=1) as pwork,
            ):
                deferred = []

                def pop_deferred():
                    if deferred:
                        deferred.pop(0)()

                def st_group(isl, jt, pt2):
                    """S^T matmuls + exp for key tile jt into pt2[:, jt%2, :]."""
                    s_ps = psum3.tile([P, 512], F32, tag="s", bufs=2)
                    isl_sl = slice(isl * 512, (isl + 1) * 512)
                    js, jc = jt // 4, (jt % 4) * P
                    for kt in range(0, CT, 2):
                        nc.tensor.matmul(
                            s_ps,
                            X[:, js, kt : kt + 2, jc : jc + P],
                            QT[:, kt : kt + 2, isl_sl],
                            start=(kt == 0), stop=(kt == 2),
                            perf_mode=DR,
                        )
                    nc.scalar.activation(
                        out=pt2[:, jt % 2, :], in_=s_ps,
                        func=mybir.ActivationFunctionType.Exp,
                        scale=SCL / WS, bias=neg2,
                    )

                def emit_znorm(isl, z_ps, u_list):
                    """zinv per q-partition from the ones-matmul Z row.

                    Pipelined per ib block: reciprocal + normalize fire as
                    soon as that block's Z lands, so the first transpose can
                    start ~1us earlier in the tail.
                    """
                    zrow = pwork.tile([1, 512], F32, tag="zrow", bufs=2)
                    nc.vector.tensor_copy(out=zrow, in_=z_ps[0:1, :])
                    zvals = pwork.tile([P, 4], F32, tag="zv", bufs=2)
                    zinv = pwork.tile([P, 4], F32, tag="zi", bufs=2)
                    osbs = []
                    for ib in range(4):
                        zx = psum3.tile([P, 1], F32, tag="t", bufs=1)
                        nc.tensor.matmul(
                            zx, zrow[:, ib * P : (ib + 1) * P], onef,
                            start=True, stop=True,
                        )
                        nc.vector.tensor_copy(out=zvals[:, ib : ib + 1], in_=zx)
                        nc.vector.reciprocal(
                            out=zinv[:, ib : ib + 1], in_=zvals[:, ib : ib + 1]
                        )
                        osb = pwork.tile([P, C], FP8, tag="osb", bufs=4)
                        nc.vector.tensor_scalar(
                            out=osb, in0=u_list[ib],
                            scalar1=zinv[:, ib : ib + 1], scalar2=OS / WS,
                            op0=mybir.AluOpType.mult, op1=mybir.AluOpType.mult,
                        )
                        osbs.append(osb)
                    return osbs

                def otr_closures(isl, osbs):
                    """Deferred: packed-word transpose of O^T blocks into QP.

                    osb fp8 [q, 512c] viewed as f32r words [q, 128w] (word w
                    = channels 4w..4w+3); one transpose per ib block.
                    """
                    ops = []
                    for ib in range(4):
                        def otr(ib=ib):
                            t_ps = psum3.tile([P, P], F32, tag="t", bufs=1)
                            nc.tensor.transpose(
                                t_ps, osbs[ib].bitcast(F32), idr
                            )
                            nc.vector.tensor_copy(
                                out=QP[:, isl, ib, :], in_=t_ps
                            )

                        ops.append(otr)
                    return ops

                def proj_group(h, ob):
                    """Final projection for output block ob over i-slice h.

                    rhs: QP packed words re-read as fp8 [p, r, ib, q]; DR
                    pairs r in {0,1} and {2,3}; out cols ordered (ib, q) =
                    natural q. Then 1/512 unscale + residual + store.
                    """
                    sl = slice(h * 512, (h + 1) * 512)
                    pr = psum3.tile([P, 512], F32, tag="t", bufs=1)
                    rhs4 = QP[:, h, :, :].bitcast(FP8).rearrange(
                        "p ib (q r) -> p r ib q", r=4
                    )
                    for rp in range(2):
                        nc.tensor.matmul(
                            pr,
                            wppk[:, 2 * rp : 2 * rp + 2, ob * P : (ob + 1) * P],
                            rhs4[:, 2 * rp : 2 * rp + 2, :, :],
                            start=(rp == 0), stop=(rp == 1),
                            perf_mode=DR,
                        )
                    ost = pwork.tile([P, 512], F32, tag="ost", bufs=3)
                    for hh in range(2):
                        cs = slice(hh * 256, (hh + 1) * 256)
                        gs2 = slice(h * 512 + hh * 256, h * 512 + (hh + 1) * 256)
                        nc.vector.scalar_tensor_tensor(
                            out=ost[:, cs], in0=pr[:, cs], scalar=1.0 / (WS * OS),
                            in1=XQB[:, ob, gs2], op0=mybir.AluOpType.mult,
                            op1=mybir.AluOpType.add,
                        )
                        nc.sync.dma_start(out=OUT_d[:, ob, gs2], in_=ost[:, cs])

                for isl in range(2):
                    u_list = [
                        psum3.tile([P, C], F32, tag=f"u{ib}", bufs=1, name=f"u{ib}")
                        for ib in range(4)
                    ]
                    z_ps = psum3.tile([P, 512], F32, tag="z", bufs=1, name="z")

                    def pv_z(tpair, pair_pt2, u_list=u_list, z_ps=z_ps):
                        for ib in range(4):
                            nc.tensor.matmul(
                                u_list[ib],
                                pair_pt2[:, :, ib * P : (ib + 1) * P],
                                VT[:, 2 * tpair : 2 * tpair + 2, :],
                                start=(tpair == 0), stop=(tpair == 15),
                                perf_mode=DR,
                            )
                        nc.tensor.matmul(
                            z_ps, onesf8, pair_pt2,
                            start=(tpair == 0), stop=(tpair == 15),
                            perf_mode=DR,
                        )

                    pending = None  # deferred (tpair, pt2): PV trails the exp
                    pt2 = pwork.tile([P, 2, 512], FP8, tag="p", bufs=4)
                    st_group(isl, 0, pt2)
                    for jt in range(NJT):
                        cur_pt2 = pt2
                        if jt + 1 < NJT:
                            if jt % 2 == 1:
                                pt2 = pwork.tile([P, 2, 512], FP8, tag="p", bufs=4)
                            st_group(isl, jt + 1, pt2)
                        # PV + Z one pair behind, so exp(jt) hides under PE work
                        if jt % 2 == 1:
                            if pending is not None:
                                pv_z(*pending)
                            pending = (jt // 2, cur_pt2)
                        pop_deferred()
                        # i-slice 0's O is final once its 4 transposes popped
                        # (by jt=4 of isl 1) — run the h=0 projection here.
                        if isl == 1 and jt >= 5 and (jt - 5) % 8 == 0:
                            proj_group(0, (jt - 5) // 8)
                    pv_z(*pending)
                    osbs = emit_znorm(isl, z_ps, u_list)
                    deferred.extend(otr_closures(isl, osbs))

                # ---------------- Phase 4: remaining projection (h=1) --------
                while deferred:
                    pop_deferred()
                for ob in range(CT):
                    proj_group(1, ob)

    nc.compile()
    return nc


def _get_nc():
    if "nc" not in _cached:
        _cached["nc"] = _build_program()
    return _cached["nc"]


def _make_in_maps(x, norm_gamma, norm_beta, wq, bq, wk, bk, wv, bv, wp, bp):
    gm = np.zeros((P, CT, NGROUPS), np.float32)
    em = np.zeros((NGROUPS, CT, P), np.float32)
    for t in range(CT):
        for p in range(P):
            g = (t * P + p) // GSIZE
            gm[p, t, g] = 1.0
            em[g, t, p] = 1.0

    bpe = np.asarray(bp) + np.asarray(wp) @ np.asarray(bv)  # [C]
    # packed projection weights: wppk[cw, rr, o] = 16*wp[o, 4*cw+rr]
    wppk = np.ascontiguousarray(
        (np.asarray(wp).T * WS).reshape(P, 4, C), dtype=np.float32
    )
    common = {
        "wqt": _cmaj(np.asarray(wq).T * WS, C).astype(NPFP8),
        "wk2": _cmaj(np.asarray(wk) * WS, C).astype(NPFP8),
        "wvt": _cmaj(np.asarray(wv).T * WS, C).astype(NPFP8),
        "wpt": wppk.astype(NPFP8),
        "cvec": np.stack(
            [
                _ct_layout(np.asarray(bq) * WS),
                _ct_layout(np.asarray(norm_gamma)),
                _ct_layout(np.asarray(norm_beta)),
            ],
            axis=1,
        ),
        "gmat": gm,
        "emat": em,
        "idr": np.eye(P, dtype=np.float32),
    }

    in_maps = []
    for c in range(NCORES):
        b, qi = c // 4, c % 4
        xb = np.asarray(x[b], dtype=np.float32).reshape(C, N)
        xp = np.concatenate([xb[:, qi * NQ :], xb[:, : qi * NQ]], axis=1)
        m = dict(common)
        xcm = _cmaj(xp, N).reshape(P, CT, NS, 512)
        m["xin"] = np.ascontiguousarray(
            xcm.transpose(0, 2, 1, 3).reshape(P, NS, CT * 512)
        ).astype(NPFP8)
        m["xqb"] = _cmaj(
            xb[:, qi * NQ : (qi + 1) * NQ] + bpe[:, None], NQ
        )
        in_maps.append(m)
    return in_maps


def _assemble(results):
    out = np.empty((B, C, N), np.float32)
    for c in range(NCORES):
        b, qi = c // 4, c % 4
        r = results[c]["out"]  # [P, CT, NQ]
        out[b, :, qi * NQ : (qi + 1) * NQ] = (
            r.transpose(1, 0, 2).reshape(C, NQ)
        )
    return out.reshape(B, C, HW, HW)


def _run(inputs, trace=False, trace_kwargs=None):
    nc = _get_nc()
    in_maps = _make_in_maps(**inputs)
    res = run_bass_kernel_spmd(
        nc, in_maps, list(range(NCORES)), trace=trace,
        **(trace_kwargs or {}),
    )
    return res


def kernel(**inputs):
    res = _run(inputs)
    return _assemble(res.results)
